# revision 2
# baseline (speedup 1.0000x reference)
"""Causal GQA attention (prefill) on 8 TRN2 NeuronCores.

Problem: B=2, S=2048, H=32 query heads, Hk=8 kv heads, D=128, f32 I/O.
Sharding: tensor-parallel over heads -- core c gets query heads [4c, 4c+4)
and kv head c. Attention is fully independent per head: no collectives.

Per-core kernel: 8 instances of causal attention, one per (batch, qhead).
Engine budget drives the design (ScalarE exp and PE matmul are the two
near-saturated engines):
  - Q/K are pre-cast to bf16 on the host and transposed into [d, s]
    layout by DMA-XBAR transposes straight from DRAM -- zero PE/DVE cost
    for layout (the old kernel burned ~16 PE transposes + 16 DVE copies
    per instance on this).
  - QK^T is computed per (query-superblock 512, key-block 128) into a
    6-bank PSUM ring; exp runs on ScalarE as ONE activation per group of
    up to 4 key blocks (multi-bank read) to amortize the ~0.35us fixed
    cost per activation. Diagonal (ragged) blocks are packed into
    2.5 banks so no garbage columns are ever exp'd.
  - P^T tiles feed PV matmuls as stationary weights; V carries an
    appended ones-column so the softmax denominator accumulates in the
    same PSUM tile (column 128). The 4 PV output slabs (129 cols each)
    are packed 2-per-bank into 2 PSUM banks via first-write start /
    last-write stop flags.
  - out = PV / denom via DVE reciprocal + per-partition scalar mul.
"""

import numpy as np
import ml_dtypes

import concourse.bass as bass
import concourse.tile as tile
from concourse import bacc, mybir
from concourse.bass import ts
from concourse.bass_utils import run_bass_kernel_spmd
from concourse.masks import make_upper_triangular

B = 2
S = 2048
H = 32
HK = 8
D = 128
NCORES = 8
GH = H // NCORES  # query heads per core (= group size here)
SCALE = 0.08838834764831845  # 1/sqrt(128)

F32 = mybir.dt.float32
BF16 = mybir.dt.bfloat16

NQB = S // 128  # 16 query/key blocks of 128
NSB = 4  # query superblocks of 512
RING = 6  # PSUM banks in the QK score ring
GMAX = 4  # max key-blocks per exp activation group


def build_nc() -> bass.Bass:
    nc = bacc.Bacc(
        "TRN2", target_bir_lowering=False, debug=False, num_devices=NCORES
    )
    q_d = nc.declare_dram_parameter("query", [B, S, GH, D], BF16, isOutput=False)
    k_d = nc.declare_dram_parameter("key", [B, S, 1, D], BF16, isOutput=False)
    v_d = nc.declare_dram_parameter("value", [B, S, 1, D], BF16, isOutput=False)
    o_d = nc.declare_dram_parameter("out", [B, S, GH, D], F32, isOutput=True)

    with tile.TileContext(nc) as tc:
        with (
            tc.tile_pool(name="consts", bufs=1) as consts,
            tc.tile_pool(name="pt", bufs=4) as pt_pool,
            tc.tile_pool(name="ptd", bufs=2) as ptd_pool,
            tc.tile_pool(name="osb", bufs=3) as osb_pool,
            tc.tile_pool(name="small", bufs=8) as small_pool,
            tc.tile_pool(name="psum", bufs=1, space="PSUM") as psum_pool,
        ):
            # mask[k, q] = 1 where q >= k (keep), 0 above -> kills k > q.
            mask = consts.tile([128, 128], BF16)
            make_upper_triangular(nc, mask, val=1.0, diag=True)

            kt_all = consts.tile([128, B, S], BF16)  # [d, b, k]
            qt_all = consts.tile([128, B * GH, S], BF16)  # [d, inst, q]
            v_ext = consts.tile([128, B, NQB, 132], BF16)  # [k, b, kblk, d+1]
            nc.vector.memset(v_ext[:, :, :, 128:129], 1.0)

            for b in range(B):
                for blk in range(NQB):
                    nc.sync.dma_start_transpose(
                        kt_all[:, b, ts(blk, 128)],
                        k_d[b, ts(blk, 128), 0, :],
                    )
                nc.sync.dma_start(
                    out=v_ext[:, b, :, 0:128],
                    in_=v_d[b, :, 0, :].rearrange("(n p) d -> p n d", p=128),
                )

            def load_qt(i):
                bb, gg = divmod(i, GH)
                for blk in range(NQB):
                    nc.sync.dma_start_transpose(
                        qt_all[:, i, ts(blk, 128)],
                        q_d[bb, ts(blk, 128), gg, :],
                    )

            load_qt(0)

            # PSUM: 6-bank QK ring + 2-bank packed PV accumulators
            ring = psum_pool.tile([128, RING, 512], F32)
            po = psum_pool.tile([128, 2, 512], F32)

            rp = [0]  # ring pointer

            def phase_attn(inst):
                b, g = divmod(inst, GH)
                q0 = 512  # base col into qt_all for current sq (set below)

                for sq in range(NSB):
                    q0 = 512 * sq
                    noff = 4 * sq  # off-diagonal key blocks

                    # --- plan off-diagonal groups (contiguous ring slots)
                    groups = []  # (kind, [(ki, slot)], ...)
                    ki = 0
                    while ki < noff:
                        take = min(GMAX, noff - ki, RING - rp[0] % RING)
                        s0 = rp[0] % RING
                        groups.append(
                            ("off", [(ki + t, s0 + t) for t in range(take)])
                        )
                        rp[0] += take
                        ki += take
                    # --- plan diagonal group: d0,d0+1 adjacent + d2 anywhere
                    if rp[0] % RING == RING - 1:
                        d2, d0, d1 = RING - 1, 0, 1
                        rp[0] += 3
                    else:
                        d0 = rp[0] % RING
                        d1, d2 = d0 + 1, (d0 + 2) % RING
                        rp[0] += 3
                    groups.append(("diag", (d0, d1, d2)))

                    # --- po slab write bookkeeping (per j): totals & counts
                    totals = [noff + 1 + j for j in range(4)]  # writes per j
                    bank_tot = [totals[0] + totals[1], totals[2] + totals[3]]
                    bank_cnt = [0, 0]

                    def pv_mm(pt_ap, ki, j):
                        bk = j // 2
                        off = (j % 2) * 256
                        bank_cnt[bk] += 1
                        nc.tensor.matmul(
                            po[:, bk, off : off + 129],
                            lhsT=pt_ap,
                            rhs=v_ext[:, b, ki, 0:129],
                            start=(bank_cnt[bk] == 1),
                            stop=(bank_cnt[bk] == bank_tot[bk]),
                        )

                    # --- emit
                    pts = []  # (kind, group, pt_tile)
                    for gi, grp in enumerate(groups):
                        if grp[0] == "off":
                            kis = grp[1]
                            n = len(kis)
                            s0 = kis[0][1]
                            for kk, slot in kis:
                                nc.tensor.matmul(
                                    ring[:, slot, :],
                                    lhsT=kt_all[:, b, ts(kk, 128)],
                                    rhs=qt_all[:, inst, q0 : q0 + 512],
                                    start=True,
                                    stop=True,
                                )
                            pt = pt_pool.tile([128, GMAX, 512], BF16)
                            nc.scalar.activation(
                                pt[:, 0:n, :],
                                ring[:, s0 : s0 + n, :],
                                mybir.ActivationFunctionType.Exp,
                                scale=SCALE,
                            )
                            pts.append(("off", kis, pt))
                        else:
                            d0, d1, d2 = grp[1]
                            kd = 4 * sq
                            # ki kd+0: full 512 cols
                            nc.tensor.matmul(
                                ring[:, d0, :],
                                lhsT=kt_all[:, b, ts(kd, 128)],
                                rhs=qt_all[:, inst, q0 : q0 + 512],
                                start=True, stop=True,
                            )
                            # ki kd+1 (384 cols) and kd+3 (128 cols) share d1
                            nc.tensor.matmul(
                                ring[:, d1, 0:384],
                                lhsT=kt_all[:, b, ts(kd + 1, 128)],
                                rhs=qt_all[:, inst, q0 + 128 : q0 + 512],
                                start=True, stop=False,
                            )
                            nc.tensor.matmul(
                                ring[:, d1, 384:512],
                                lhsT=kt_all[:, b, ts(kd + 3, 128)],
                                rhs=qt_all[:, inst, q0 + 384 : q0 + 512],
                                start=False, stop=True,
                            )
                            # ki kd+2: 256 cols
                            nc.tensor.matmul(
                                ring[:, d2, 0:256],
                                lhsT=kt_all[:, b, ts(kd + 2, 128)],
                                rhs=qt_all[:, inst, q0 + 256 : q0 + 512],
                                start=True, stop=True,
                            )
                            ptd = ptd_pool.tile([128, 3, 512], BF16)
                            nc.scalar.activation(
                                ptd[:, 0:2, :],
                                ring[:, d0 : d0 + 2, :],
                                mybir.ActivationFunctionType.Exp,
                                scale=SCALE,
                            )
                            nc.scalar.activation(
                                ptd[:, 2, 0:256],
                                ring[:, d2, 0:256],
                                mybir.ActivationFunctionType.Exp,
                                scale=SCALE,
                            )
                            # causal triangle masks (4 diag blocks)
                            for sl in (
                                ptd[:, 0, 0:128],
                                ptd[:, 1, 0:128],
                                ptd[:, 1, 384:512],
                                ptd[:, 2, 0:128],
                            ):
                                nc.vector.tensor_mul(sl, sl, mask)
                            pts.append(("diag", grp[1], ptd))

                        # PV for the previous group (1-group lookahead)
                        if len(pts) >= 2:
                            emit_pv(pts[-2], pv_mm, 4 * sq)
                    emit_pv(pts[-1], pv_mm, 4 * sq)

                    # --- normalize + store
                    o_sb = osb_pool.tile([128, 4, 128], F32)
                    for j in range(4):
                        bk, off = j // 2, (j % 2) * 256
                        recip = small_pool.tile([128, 1], F32)
                        nc.vector.reciprocal(
                            recip, po[:, bk, off + 128 : off + 129]
                        )
                        nc.vector.tensor_scalar_mul(
                            o_sb[:, j, :], po[:, bk, off : off + 128], recip
                        )
                    nc.sync.dma_start(
                        out=o_d[b, q0 : q0 + 512, g, :].rearrange(
                            "(n p) d -> p n d", p=128
                        ),
                        in_=o_sb,
                    )

            def emit_pv(entry, pv_mm, kd):
                kind = entry[0]
                if kind == "off":
                    kis, pt = entry[1], entry[2]
                    for t, (kk, _slot) in enumerate(kis):
                        for j in range(4):
                            pv_mm(pt[:, t, ts(j, 128)], kk, j)
                else:
                    ptd = entry[2]
                    # ki kd+0: all 4 j
                    for j in range(4):
                        pv_mm(ptd[:, 0, ts(j, 128)], kd, j)
                    # ki kd+1: j=1..3 at cols 128(j-1)
                    for j in range(1, 4):
                        pv_mm(ptd[:, 1, ts(j - 1, 128)], kd + 1, j)
                    # ki kd+2: j=2,3 at cols 0/128
                    for j in range(2, 4):
                        pv_mm(ptd[:, 2, ts(j - 2, 128)], kd + 2, j)
                    # ki kd+3: j=3
                    pv_mm(ptd[:, 1, 384:512], kd + 3, 3)

            for inst in range(B * GH):
                if inst + 1 < B * GH:
                    load_qt(inst + 1)
                phase_attn(inst)

    nc.finalize()
    return nc


def _to_bf16(x):
    return np.asarray(x, dtype=np.float32).astype(ml_dtypes.bfloat16)


def make_in_maps(query, key, value):
    qb = _to_bf16(query)
    kb = _to_bf16(key)
    vb = _to_bf16(value)
    in_maps = []
    for c in range(NCORES):
        in_maps.append(
            {
                "query": np.ascontiguousarray(qb[:, :, GH * c : GH * (c + 1), :]),
                "key": np.ascontiguousarray(kb[:, :, c : c + 1, :]),
                "value": np.ascontiguousarray(vb[:, :, c : c + 1, :]),
            }
        )
    return in_maps


def kernel(query, key, value):
    nc = build_nc()
    res = run_bass_kernel_spmd(
        nc, make_in_maps(query, key, value), core_ids=list(range(NCORES))
    )
    outs = [np.asarray(res.results[c]["out"]) for c in range(NCORES)]
    return np.concatenate(outs, axis=2).astype(np.float32)


if __name__ == "__main__":
    rng = np.random.default_rng(0)
    q = rng.standard_normal((B, S, H, D), dtype=np.float32)
    k = rng.standard_normal((B, S, HK, D), dtype=np.float32)
    v = rng.standard_normal((B, S, HK, D), dtype=np.float32)
    out = kernel(q, k, v)
    print("out", out.shape, out.dtype, float(np.abs(out).max()))


# revision 4
# speedup vs baseline: 1.4125x; 1.4125x over previous
"""Causal GQA attention (prefill) on 8 TRN2 NeuronCores.

Problem: B=2, S=2048, H=32 query heads, Hk=8 kv heads, D=128, f32 I/O.
Sharding: tensor-parallel over heads -- core c gets query heads [4c, 4c+4)
and kv head c. Attention is fully independent per head: no collectives.

Per-core kernel: 8 instances of causal attention, one per (batch, qhead).
Engine budget drives the design (ScalarE exp and PE matmul are the two
near-saturated engines):
  - Q/K/V are pre-cast to bf16 on the host (halves DMA bytes, kills all
    on-device casts). Q^T/K^T layouts are produced by PE transposes
    (bf16 transpose-mode matmuls are LDWEIGHTS-free) whose outputs are
    packed 4-8 blocks per PSUM bank tail and copied back to SBUF in one
    wide DVE copy instead of per-block copies.
  - QK^T is computed per (query-superblock 512, key-block 128) into a
    6-bank PSUM ring; exp runs on ScalarE as ONE activation per group of
    up to 4 key blocks (multi-bank PSUM read) to amortize the ~0.35us
    fixed cost per activation. Diagonal (ragged) blocks are packed into
    2.5 banks so no garbage columns are ever exp'd.
  - P^T tiles feed PV matmuls as stationary weights; V carries an
    appended ones-column so the softmax denominator accumulates in the
    same PSUM tile (column 128). The 4 PV output slabs (129 cols each)
    are packed 2-per-bank into 2 PSUM banks via first-write start /
    last-write stop flags.
  - out = PV / denom via DVE reciprocal + per-partition scalar mul.
"""

import numpy as np
import ml_dtypes

import concourse.bass as bass
import concourse.tile as tile
from concourse import bacc, mybir
from concourse.bass import ts
from concourse.bass_utils import run_bass_kernel_spmd
from concourse.masks import make_identity, make_upper_triangular

B = 2
S = 2048
H = 32
HK = 8
D = 128
NCORES = 8
GH = H // NCORES  # query heads per core (= group size here)
SCALE = 0.08838834764831845  # 1/sqrt(128)

F32 = mybir.dt.float32
BF16 = mybir.dt.bfloat16

NQB = S // 128  # 16 query/key blocks of 128
NSB = 4  # query superblocks of 512
RING = 6  # PSUM banks in the QK score ring
GMAX = 4  # max key-blocks per exp activation group


def build_nc() -> bass.Bass:
    nc = bacc.Bacc(
        "TRN2", target_bir_lowering=False, debug=False, num_devices=NCORES
    )
    q_d = nc.declare_dram_parameter("query", [B, S, GH, D], BF16, isOutput=False)
    k_d = nc.declare_dram_parameter("key", [B, S, 1, D], BF16, isOutput=False)
    v_d = nc.declare_dram_parameter("value", [B, S, 1, D], BF16, isOutput=False)
    o_d = nc.declare_dram_parameter("out", [B, S, GH, D], F32, isOutput=True)

    with tile.TileContext(nc) as tc:
        with (
            tc.tile_pool(name="consts", bufs=1) as consts,
            tc.tile_pool(name="nat", bufs=4) as nat_pool,
            tc.tile_pool(name="pt", bufs=4) as pt_pool,
            tc.tile_pool(name="ptd", bufs=2) as ptd_pool,
            tc.tile_pool(name="osb", bufs=3) as osb_pool,
            tc.tile_pool(name="small", bufs=8) as small_pool,
            tc.tile_pool(name="psum", bufs=1, space="PSUM") as psum_pool,
        ):
            ident_bf = consts.tile([128, 128], BF16)
            make_identity(nc, ident_bf)
            # mask[k, q] = 1 where q >= k (keep), 0 above -> kills k > q.
            mask = consts.tile([128, 128], BF16)
            make_upper_triangular(nc, mask, val=1.0, diag=True)

            kt_all = consts.tile([128, B, S], BF16)  # [d, b, k]
            qt_all = consts.tile([128, B * GH, S], BF16)  # [d, inst, q]
            v_ext = consts.tile([128, B, NQB, 132], BF16)  # [k, b, kblk, d+1]
            nc.vector.memset(v_ext[:, :, :, 128:129], 1.0)
            for b in range(B):
                nc.sync.dma_start(
                    out=v_ext[:, b, :, 0:128],
                    in_=v_d[b, :, 0, :].rearrange("(n p) d -> p n d", p=128),
                )

            # PSUM: 6-bank QK ring + 2-bank packed PV accumulators
            ring = psum_pool.tile([128, RING, 512], F32)
            po = psum_pool.tile([128, 2, 512], F32)

            def bank_bf(bk, blk8):
                # 128x128 bf16 view of bank bk, block slot blk8 (0..7)
                return ring[:, bk, 64 * blk8 : 64 * (blk8 + 1)].bitcast(BF16)

            def load_nat(src_ap):
                t = nat_pool.tile([128, NQB, 128], BF16, tag="nat")
                nc.sync.dma_start(
                    out=t, in_=src_ap.rearrange("(n p) d -> p n d", p=128)
                )
                return t

            # ---- startup: K (2x16 blocks) + Q inst0 (16) via PE transposes
            # packed 8 blocks per ring bank, one wide DVE copy per bank.
            k_nats = [load_nat(k_d[b, :, 0, :]) for b in range(B)]
            q_nat0 = load_nat(q_d[0, :, 0, :])
            startup = [  # (bank, src nat tile, dst stripe, dst col base)
                (0, k_nats[0], 0, kt_all[:, 0, 0:1024]),
                (4, q_nat0, 0, qt_all[:, 0, 0:1024]),
                (1, k_nats[0], 8, kt_all[:, 0, 1024:2048]),
                (5, q_nat0, 8, qt_all[:, 0, 1024:2048]),
                (2, k_nats[1], 0, kt_all[:, 1, 0:1024]),
                (3, k_nats[1], 8, kt_all[:, 1, 1024:2048]),
            ]
            for bk, nat, blk0, dst in startup:
                for i in range(8):
                    nc.tensor.transpose(
                        bank_bf(bk, i), nat[:, blk0 + i, :], ident_bf
                    )
                nc.vector.tensor_copy(
                    dst, ring[:, bk, :].bitcast(BF16)
                )

            rp = [0]  # ring pointer

            def phase_attn(inst, q_nat_next):
                b, g = divmod(inst, GH)
                nxt = inst + 1

                for sq in range(NSB):
                    q0 = 512 * sq
                    noff = 4 * sq  # off-diagonal key blocks

                    # --- plan off-diagonal groups (contiguous ring slots)
                    plans = []
                    ki = 0
                    while ki < noff:
                        take = min(GMAX, noff - ki, RING - rp[0] % RING)
                        s0 = rp[0] % RING
                        plans.append(("off", [(ki + t, s0 + t) for t in range(take)]))
                        rp[0] += take
                        ki += take
                    # --- plan diagonal group: d0,d0+1 adjacent + d2 anywhere
                    if rp[0] % RING == RING - 1:
                        d2, d0, d1 = RING - 1, 0, 1
                    else:
                        d0 = rp[0] % RING
                        d1, d2 = d0 + 1, (d0 + 2) % RING
                    rp[0] += 3
                    plans.append(("diag", (d0, d1, d2)))

                    # --- drip next instance's Q transposes into d2's tail
                    # (cols 256:512 f32 = 4 bf16 blocks; QK/ACT use 0:256)
                    if q_nat_next is not None:
                        for i in range(4):
                            nc.tensor.transpose(
                                ring[:, d2, 256 + 64 * i : 320 + 64 * i].bitcast(BF16),
                                q_nat_next[:, 4 * sq + i, :],
                                ident_bf,
                            )
                        nc.vector.tensor_copy(
                            qt_all[:, nxt, q0 : q0 + 512],
                            ring[:, d2, 256:512].bitcast(BF16),
                        )

                    # --- po slab write bookkeeping
                    totals = [noff + 1 + j for j in range(4)]
                    bank_tot = [totals[0] + totals[1], totals[2] + totals[3]]
                    bank_cnt = [0, 0]

                    def pv_mm(pt_ap, kk, j):
                        bk = j // 2
                        off = (j % 2) * 256
                        bank_cnt[bk] += 1
                        nc.tensor.matmul(
                            po[:, bk, off : off + 129],
                            lhsT=pt_ap,
                            rhs=v_ext[:, b, kk, 0:129],
                            start=(bank_cnt[bk] == 1),
                            stop=(bank_cnt[bk] == bank_tot[bk]),
                        )

                    # --- emit QK/ACT/mask per group, PV with 1-group lookahead
                    pts = []
                    for grp in plans:
                        if grp[0] == "off":
                            kis = grp[1]
                            n = len(kis)
                            s0 = kis[0][1]
                            for kk, slot in kis:
                                nc.tensor.matmul(
                                    ring[:, slot, :],
                                    lhsT=kt_all[:, b, ts(kk, 128)],
                                    rhs=qt_all[:, inst, q0 : q0 + 512],
                                    start=True,
                                    stop=True,
                                )
                            pt = pt_pool.tile([128, GMAX, 512], BF16)
                            nc.scalar.activation(
                                pt[:, 0:n, :],
                                ring[:, s0 : s0 + n, :],
                                mybir.ActivationFunctionType.Exp,
                                scale=SCALE,
                            )
                            pts.append(("off", kis, pt))
                        else:
                            d0, d1, d2 = grp[1]
                            kd = 4 * sq
                            nc.tensor.matmul(
                                ring[:, d0, :],
                                lhsT=kt_all[:, b, ts(kd, 128)],
                                rhs=qt_all[:, inst, q0 : q0 + 512],
                                start=True, stop=True,
                            )
                            nc.tensor.matmul(
                                ring[:, d1, 0:384],
                                lhsT=kt_all[:, b, ts(kd + 1, 128)],
                                rhs=qt_all[:, inst, q0 + 128 : q0 + 512],
                                start=True, stop=False,
                            )
                            nc.tensor.matmul(
                                ring[:, d1, 384:512],
                                lhsT=kt_all[:, b, ts(kd + 3, 128)],
                                rhs=qt_all[:, inst, q0 + 384 : q0 + 512],
                                start=False, stop=True,
                            )
                            nc.tensor.matmul(
                                ring[:, d2, 0:256],
                                lhsT=kt_all[:, b, ts(kd + 2, 128)],
                                rhs=qt_all[:, inst, q0 + 256 : q0 + 512],
                                start=True, stop=True,
                            )
                            ptd = ptd_pool.tile([128, 3, 512], BF16)
                            nc.scalar.activation(
                                ptd[:, 0:2, :],
                                ring[:, d0 : d0 + 2, :],
                                mybir.ActivationFunctionType.Exp,
                                scale=SCALE,
                            )
                            nc.scalar.activation(
                                ptd[:, 2, 0:256],
                                ring[:, d2, 0:256],
                                mybir.ActivationFunctionType.Exp,
                                scale=SCALE,
                            )
                            for sl in (
                                ptd[:, 0, 0:128],
                                ptd[:, 1, 0:128],
                                ptd[:, 1, 384:512],
                                ptd[:, 2, 0:128],
                            ):
                                nc.vector.tensor_mul(sl, sl, mask)
                            pts.append(("diag", grp[1], ptd))

                        if len(pts) >= 2:
                            emit_pv(pts[-2], pv_mm, 4 * sq)
                    emit_pv(pts[-1], pv_mm, 4 * sq)

                    # --- normalize + store
                    o_sb = osb_pool.tile([128, 4, 128], F32)
                    for j in range(4):
                        bk, off = j // 2, (j % 2) * 256
                        recip = small_pool.tile([128, 1], F32)
                        nc.vector.reciprocal(
                            recip, po[:, bk, off + 128 : off + 129]
                        )
                        nc.vector.tensor_scalar_mul(
                            o_sb[:, j, :], po[:, bk, off : off + 128], recip
                        )
                    nc.sync.dma_start(
                        out=o_d[b, q0 : q0 + 512, g, :].rearrange(
                            "(n p) d -> p n d", p=128
                        ),
                        in_=o_sb,
                    )

            def emit_pv(entry, pv_mm, kd):
                if entry[0] == "off":
                    kis, pt = entry[1], entry[2]
                    for t, (kk, _slot) in enumerate(kis):
                        for j in range(4):
                            pv_mm(pt[:, t, ts(j, 128)], kk, j)
                else:
                    ptd = entry[2]
                    for j in range(4):
                        pv_mm(ptd[:, 0, ts(j, 128)], kd, j)
                    for j in range(1, 4):
                        pv_mm(ptd[:, 1, ts(j - 1, 128)], kd + 1, j)
                    for j in range(2, 4):
                        pv_mm(ptd[:, 2, ts(j - 2, 128)], kd + 2, j)
                    pv_mm(ptd[:, 1, 384:512], kd + 3, 3)

            for inst in range(B * GH):
                if inst + 1 < B * GH:
                    bn, gn = divmod(inst + 1, GH)
                    q_nat_next = load_nat(q_d[bn, :, gn, :])
                else:
                    q_nat_next = None
                phase_attn(inst, q_nat_next)

    nc.finalize()
    return nc


def _to_bf16(x):
    return np.asarray(x, dtype=np.float32).astype(ml_dtypes.bfloat16)


def make_in_maps(query, key, value):
    qb = _to_bf16(query)
    kb = _to_bf16(key)
    vb = _to_bf16(value)
    in_maps = []
    for c in range(NCORES):
        in_maps.append(
            {
                "query": np.ascontiguousarray(qb[:, :, GH * c : GH * (c + 1), :]),
                "key": np.ascontiguousarray(kb[:, :, c : c + 1, :]),
                "value": np.ascontiguousarray(vb[:, :, c : c + 1, :]),
            }
        )
    return in_maps


def kernel(query, key, value):
    nc = build_nc()
    res = run_bass_kernel_spmd(
        nc, make_in_maps(query, key, value), core_ids=list(range(NCORES))
    )
    outs = [np.asarray(res.results[c]["out"]) for c in range(NCORES)]
    return np.concatenate(outs, axis=2).astype(np.float32)


if __name__ == "__main__":
    rng = np.random.default_rng(0)
    q = rng.standard_normal((B, S, H, D), dtype=np.float32)
    k = rng.standard_normal((B, S, HK, D), dtype=np.float32)
    v = rng.standard_normal((B, S, HK, D), dtype=np.float32)
    out = kernel(q, k, v)
    print("out", out.shape, out.dtype, float(np.abs(out).max()))


# revision 5
# speedup vs baseline: 1.5235x; 1.0786x over previous
"""Causal GQA attention (prefill) on 8 TRN2 NeuronCores.

Problem: B=2, S=2048, H=32 query heads, Hk=8 kv heads, D=128, f32 I/O.
Sharding: tensor-parallel over heads -- core c gets query heads [4c, 4c+4)
and kv head c. Attention is fully independent per head: no collectives.

Per-core kernel: 8 instances of causal attention, one per (batch, qhead),
processed as a software-pipelined stream of 32 (instance, superblock)
items. Engine budget drives the design (ScalarE exp and PE matmul are
the two near-saturated engines; the PE is strictly in-order so the
emission order IS the schedule):
  - Q/K/V are pre-cast to bf16 on the host (halves DMA bytes, kills all
    on-device casts). Q^T/K^T layouts are produced by PE transposes
    (bf16 transpose-mode matmuls are LDWEIGHTS-free) whose outputs are
    packed 4-8 blocks per PSUM bank tail and copied back to SBUF in one
    wide DVE copy instead of per-block copies.
  - QK^T is computed per (query-superblock 512, key-block 128) into a
    6-bank PSUM ring organized as two 3-bank pages; exp runs on ScalarE
    as ONE activation per page (up to 3 key blocks = 1536 cols) to
    amortize the ~0.35us fixed cost per activation. Diagonal (ragged)
    blocks are packed into 2.5 banks of one page so no garbage columns
    are ever exp'd.
  - P^T tiles feed PV matmuls as stationary weights; V carries an
    appended ones-column so the softmax denominator accumulates in the
    same PSUM tile (column 128). The 4 PV output slabs (129 cols each)
    are packed 2-per-bank into 2 PSUM banks via first-write start /
    last-write stop flags.
  - Items are phase-shifted: item n's PV matmuls are woven between item
    n+1's QK groups as in-order filler, so ring-recycle and exp-latency
    waits never idle the PE (which would also drop its p-state clock).
  - out = PV / denom via DVE reciprocal + per-partition scalar mul.
"""

import numpy as np
import ml_dtypes

import concourse.bass as bass
import concourse.tile as tile
from concourse import bacc, mybir
from concourse.bass import ts
from concourse.bass_utils import run_bass_kernel_spmd
from concourse.masks import make_identity, make_upper_triangular

B = 2
S = 2048
H = 32
HK = 8
D = 128
NCORES = 8
GH = H // NCORES  # query heads per core (= group size here)
SCALE = 0.08838834764831845  # 1/sqrt(128)

F32 = mybir.dt.float32
BF16 = mybir.dt.bfloat16

NQB = S // 128  # 16 query/key blocks of 128
NSB = 4  # query superblocks of 512
PAGE = 3  # ring page size in banks; ring = 2 pages


def build_nc() -> bass.Bass:
    nc = bacc.Bacc(
        "TRN2", target_bir_lowering=False, debug=False, num_devices=NCORES
    )
    q_d = nc.declare_dram_parameter("query", [B, S, GH, D], BF16, isOutput=False)
    k_d = nc.declare_dram_parameter("key", [B, S, 1, D], BF16, isOutput=False)
    v_d = nc.declare_dram_parameter("value", [B, S, 1, D], BF16, isOutput=False)
    o_d = nc.declare_dram_parameter("out", [B, S, GH, D], F32, isOutput=True)

    with tile.TileContext(nc) as tc:
        with (
            tc.tile_pool(name="consts", bufs=1) as consts,
            tc.tile_pool(name="nat", bufs=4) as nat_pool,
            tc.tile_pool(name="pt", bufs=10) as pt_pool,
            tc.tile_pool(name="osb", bufs=3) as osb_pool,
            tc.tile_pool(name="small", bufs=8) as small_pool,
            tc.tile_pool(name="psum", bufs=1, space="PSUM") as psum_pool,
        ):
            ident_bf = consts.tile([128, 128], BF16)
            make_identity(nc, ident_bf)
            # mask[k, q] = 1 where q >= k (keep), 0 above -> kills k > q.
            mask = consts.tile([128, 128], BF16)
            make_upper_triangular(nc, mask, val=1.0, diag=True)

            kt_all = consts.tile([128, B, S], BF16)  # [d, b, k]
            qt_all = consts.tile([128, B * GH, S], BF16)  # [d, inst, q]
            v_ext = consts.tile([128, B, NQB, 132], BF16)  # [k, b, kblk, d+1]
            nc.vector.memset(v_ext[:, :, :, 128:129], 1.0)
            for b in range(B):
                nc.sync.dma_start(
                    out=v_ext[:, b, :, 0:128],
                    in_=v_d[b, :, 0, :].rearrange("(n p) d -> p n d", p=128),
                )

            # PSUM: 6-bank QK ring (two 3-bank pages) + 2-bank packed PV acc
            ring = psum_pool.tile([128, 2 * PAGE, 512], F32)
            po = psum_pool.tile([128, 2, 512], F32)

            def load_nat(src_ap):
                t = nat_pool.tile([128, NQB, 128], BF16, tag="nat")
                nc.sync.dma_start(
                    out=t, in_=src_ap.rearrange("(n p) d -> p n d", p=128)
                )
                return t

            # ---- startup: K (2x16 blocks) + Q inst0 (16) via PE transposes
            # packed 8 blocks per ring bank, one wide DVE copy per bank.
            k_nats = [load_nat(k_d[b, :, 0, :]) for b in range(B)]
            q_nat0 = load_nat(q_d[0, :, 0, :])
            startup = [
                (0, k_nats[0], 0, kt_all[:, 0, 0:1024]),
                (4, q_nat0, 0, qt_all[:, 0, 0:1024]),
                (1, k_nats[0], 8, kt_all[:, 0, 1024:2048]),
                (5, q_nat0, 8, qt_all[:, 0, 1024:2048]),
                (2, k_nats[1], 0, kt_all[:, 1, 0:1024]),
                (3, k_nats[1], 8, kt_all[:, 1, 1024:2048]),
            ]
            for bk, nat, blk0, dst in startup:
                for i in range(8):
                    nc.tensor.transpose(
                        ring[:, bk, 64 * i : 64 * (i + 1)].bitcast(BF16),
                        nat[:, blk0 + i, :],
                        ident_bf,
                    )
                nc.vector.tensor_copy(dst, ring[:, bk, :].bitcast(BF16))

            page = [0]  # alternating ring page allocator

            def next_page():
                p = page[0]
                page[0] ^= 1
                return PAGE * p  # bank base

            # pending = (pv_closures, finish_closure) of the previous item
            pending = [None]

            def emit_pending_chunk(frac_done):
                """Emit pending PV closures up to fraction frac_done."""
                if pending[0] is None:
                    return
                pvs, _fin, cursor = pending[0]
                tgt = int(len(pvs) * frac_done + 0.5)
                while cursor[0] < tgt:
                    pvs[cursor[0]]()
                    cursor[0] += 1

            def finish_pending():
                if pending[0] is None:
                    return
                pvs, fin, cursor = pending[0]
                while cursor[0] < len(pvs):
                    pvs[cursor[0]]()
                    cursor[0] += 1
                fin()
                pending[0] = None

            def phase_item(inst, sq, q_nat_next):
                b, g = divmod(inst, GH)
                nxt = inst + 1
                q0 = 512 * sq
                noff = 4 * sq
                kd = 4 * sq

                # group plan: off-diag in chunks of <=3 on alternating
                # pages, then the diagonal on a full page.
                ngroups = (noff + PAGE - 1) // PAGE + 1

                # --- this item's PV bookkeeping (deferred emission)
                totals = [noff + 1 + j for j in range(4)]
                bank_tot = [totals[0] + totals[1], totals[2] + totals[3]]
                bank_cnt = [0, 0]
                pvs = []

                def defer_pv(pt_ap, kk, j):
                    def run(pt_ap=pt_ap, kk=kk, j=j):
                        bk = j // 2
                        off = (j % 2) * 256
                        bank_cnt[bk] += 1
                        nc.tensor.matmul(
                            po[:, bk, off : off + 129],
                            lhsT=pt_ap,
                            rhs=v_ext[:, b, kk, 0:129],
                            start=(bank_cnt[bk] == 1),
                            stop=(bank_cnt[bk] == bank_tot[bk]),
                        )
                    pvs.append(run)

                gi = 0
                ki = 0
                while ki < noff:
                    n = min(PAGE, noff - ki)
                    base = next_page()
                    for t in range(n):
                        nc.tensor.matmul(
                            ring[:, base + t, :],
                            lhsT=kt_all[:, b, ts(ki + t, 128)],
                            rhs=qt_all[:, inst, q0 : q0 + 512],
                            start=True,
                            stop=True,
                        )
                    pt = pt_pool.tile([128, PAGE, 512], BF16)
                    nc.scalar.activation(
                        pt[:, 0:n, :],
                        ring[:, base : base + n, :],
                        mybir.ActivationFunctionType.Exp,
                        scale=SCALE,
                    )
                    for t in range(n):
                        for j in range(4):
                            defer_pv(pt[:, t, ts(j, 128)], ki + t, j)
                    ki += n
                    gi += 1
                    emit_pending_chunk(gi / ngroups)

                # --- diagonal group on a full page
                base = next_page()
                d0, d1, d2 = base, base + 1, base + 2
                nc.tensor.matmul(
                    ring[:, d0, :],
                    lhsT=kt_all[:, b, ts(kd, 128)],
                    rhs=qt_all[:, inst, q0 : q0 + 512],
                    start=True, stop=True,
                )
                nc.tensor.matmul(
                    ring[:, d1, 0:384],
                    lhsT=kt_all[:, b, ts(kd + 1, 128)],
                    rhs=qt_all[:, inst, q0 + 128 : q0 + 512],
                    start=True, stop=False,
                )
                nc.tensor.matmul(
                    ring[:, d1, 384:512],
                    lhsT=kt_all[:, b, ts(kd + 3, 128)],
                    rhs=qt_all[:, inst, q0 + 384 : q0 + 512],
                    start=False, stop=True,
                )
                nc.tensor.matmul(
                    ring[:, d2, 0:256],
                    lhsT=kt_all[:, b, ts(kd + 2, 128)],
                    rhs=qt_all[:, inst, q0 + 256 : q0 + 512],
                    start=True, stop=True,
                )
                ptd = pt_pool.tile([128, PAGE, 512], BF16)
                nc.scalar.activation(
                    ptd[:, 0:2, :],
                    ring[:, d0 : d0 + 2, :],
                    mybir.ActivationFunctionType.Exp,
                    scale=SCALE,
                )
                nc.scalar.activation(
                    ptd[:, 2, 0:256],
                    ring[:, d2, 0:256],
                    mybir.ActivationFunctionType.Exp,
                    scale=SCALE,
                )
                for sl in (
                    ptd[:, 0, 0:128],
                    ptd[:, 1, 0:128],
                    ptd[:, 1, 384:512],
                    ptd[:, 2, 0:128],
                ):
                    nc.vector.tensor_mul(sl, sl, mask)

                for j in range(4):
                    defer_pv(ptd[:, 0, ts(j, 128)], kd, j)
                for j in range(1, 4):
                    defer_pv(ptd[:, 1, ts(j - 1, 128)], kd + 1, j)
                for j in range(2, 4):
                    defer_pv(ptd[:, 2, ts(j - 2, 128)], kd + 2, j)
                defer_pv(ptd[:, 1, 384:512], kd + 3, 3)

                # --- drip next instance's Q transposes into d2's tail
                # (cols 256:512 f32 = 4 bf16 blocks; QK/ACT use 0:256 only)
                if q_nat_next is not None:
                    for i in range(4):
                        nc.tensor.transpose(
                            ring[:, d2, 256 + 64 * i : 320 + 64 * i].bitcast(BF16),
                            q_nat_next[:, 4 * sq + i, :],
                            ident_bf,
                        )
                    nc.vector.tensor_copy(
                        qt_all[:, nxt, q0 : q0 + 512],
                        ring[:, d2, 256:512].bitcast(BF16),
                    )

                # --- finish previous item (its remaining PV + normalize)
                finish_pending()

                def finish():
                    o_sb = osb_pool.tile([128, 4, 128], F32)
                    for j in range(4):
                        bk, off = j // 2, (j % 2) * 256
                        recip = small_pool.tile([128, 1], F32)
                        nc.vector.reciprocal(
                            recip, po[:, bk, off + 128 : off + 129]
                        )
                        nc.vector.tensor_scalar_mul(
                            o_sb[:, j, :], po[:, bk, off : off + 128], recip
                        )
                    nc.sync.dma_start(
                        out=o_d[b, q0 : q0 + 512, g, :].rearrange(
                            "(n p) d -> p n d", p=128
                        ),
                        in_=o_sb,
                    )

                pending[0] = (pvs, finish, [0])

            for inst in range(B * GH):
                if inst + 1 < B * GH:
                    bn, gn = divmod(inst + 1, GH)
                    q_nat_next = load_nat(q_d[bn, :, gn, :])
                else:
                    q_nat_next = None
                for sq in range(NSB):
                    phase_item(inst, sq, q_nat_next if sq < NSB else None)
            finish_pending()

    nc.finalize()
    return nc


def _to_bf16(x):
    return np.asarray(x, dtype=np.float32).astype(ml_dtypes.bfloat16)


def make_in_maps(query, key, value):
    qb = _to_bf16(query)
    kb = _to_bf16(key)
    vb = _to_bf16(value)
    in_maps = []
    for c in range(NCORES):
        in_maps.append(
            {
                "query": np.ascontiguousarray(qb[:, :, GH * c : GH * (c + 1), :]),
                "key": np.ascontiguousarray(kb[:, :, c : c + 1, :]),
                "value": np.ascontiguousarray(vb[:, :, c : c + 1, :]),
            }
        )
    return in_maps


def kernel(query, key, value):
    nc = build_nc()
    res = run_bass_kernel_spmd(
        nc, make_in_maps(query, key, value), core_ids=list(range(NCORES))
    )
    outs = [np.asarray(res.results[c]["out"]) for c in range(NCORES)]
    return np.concatenate(outs, axis=2).astype(np.float32)


if __name__ == "__main__":
    rng = np.random.default_rng(0)
    q = rng.standard_normal((B, S, H, D), dtype=np.float32)
    k = rng.standard_normal((B, S, HK, D), dtype=np.float32)
    v = rng.standard_normal((B, S, HK, D), dtype=np.float32)
    out = kernel(q, k, v)
    print("out", out.shape, out.dtype, float(np.abs(out).max()))


# revision 13
# speedup vs baseline: 2.2061x; 1.4481x over previous
"""Causal GQA attention (prefill) on 8 TRN2 NeuronCores.

Problem: B=2, S=2048, H=32 query heads, Hk=8 kv heads, D=128, f32 I/O.
Sharding: tensor-parallel over heads -- core c gets query heads [4c, 4c+4)
and kv head c. Attention is fully independent per head: no collectives.

Per-core kernel: 8 instances of causal attention, one per (batch, qhead),
processed as a software-pipelined stream of 32 (instance, superblock)
items. Engine budget drives the design (ScalarE exp and PE matmul are
the two near-saturated engines; the PE is strictly in-order so the
emission order IS the schedule):
  - Q/K/V are pre-cast to bf16 on the host (halves DMA bytes, kills all
    on-device casts). Q^T/K^T layouts are produced by PE transposes
    (bf16 transpose-mode matmuls are LDWEIGHTS-free) whose outputs are
    packed 4-8 blocks per PSUM bank tail and copied back to SBUF in one
    wide DVE copy instead of per-block copies.
  - QK^T is computed per (query-superblock 512, key-block 128) into a
    6-bank PSUM ring organized as two 3-bank pages; exp runs on ScalarE
    as ONE activation per page (up to 3 key blocks = 1536 cols) to
    amortize the ~0.35us fixed cost per activation. Diagonal (ragged)
    blocks are packed into 2.5 banks of one page so no garbage columns
    are ever exp'd.
  - P^T tiles feed PV matmuls as stationary weights; V carries an
    appended ones-column so the softmax denominator accumulates in the
    same PSUM tile (column 128). The 4 PV output slabs (129 cols each)
    are packed 2-per-bank into 2 PSUM banks via first-write start /
    last-write stop flags.
  - Items are phase-shifted: item n's PV matmuls are woven between item
    n+1's QK groups as in-order filler, so ring-recycle and exp-latency
    waits never idle the PE (which would also drop its p-state clock).
  - out = PV / denom via DVE reciprocal + per-partition scalar mul.
"""

import numpy as np
import ml_dtypes

import concourse.bass as bass
import concourse.tile as tile
from concourse import bacc, mybir
from concourse.bass import ts
from concourse.bass_utils import run_bass_kernel_spmd
from concourse.masks import make_identity, make_upper_triangular

B = 2
S = 2048
H = 32
HK = 8
D = 128
NCORES = 8
GH = H // NCORES  # query heads per core (= group size here)
SCALE = 0.08838834764831845  # 1/sqrt(128)

F32 = mybir.dt.float32
BF16 = mybir.dt.bfloat16

NQB = S // 128  # 16 query/key blocks of 128
NSB = 4  # query superblocks of 512
PAGE = 3  # ring page size in banks; ring = 2 pages


def build_nc() -> bass.Bass:
    nc = bacc.Bacc(
        "TRN2", target_bir_lowering=False, debug=False, num_devices=NCORES
    )
    q_d = nc.declare_dram_parameter("query", [B, S, GH, D], BF16, isOutput=False)
    k_d = nc.declare_dram_parameter("key", [B, S, 1, D], BF16, isOutput=False)
    v_d = nc.declare_dram_parameter("value", [B, S, 1, D], BF16, isOutput=False)
    o_d = nc.declare_dram_parameter("out", [B, S, GH, D], F32, isOutput=True)

    with tile.TileContext(nc) as tc:
        with (
            tc.tile_pool(name="consts", bufs=1) as consts,
            tc.tile_pool(name="nat", bufs=4) as nat_pool,
            tc.tile_pool(name="pt", bufs=10) as pt_pool,
            tc.tile_pool(name="osb", bufs=3) as osb_pool,
            tc.tile_pool(name="small", bufs=8) as small_pool,
            tc.tile_pool(name="psum", bufs=1, space="PSUM") as psum_pool,
        ):
            ident_bf = consts.tile([128, 128], BF16)
            make_identity(nc, ident_bf)
            # mask[k, q] = 1 where q >= k (keep), 0 above -> kills k > q.
            mask = consts.tile([128, 128], BF16)
            make_upper_triangular(nc, mask, val=1.0, diag=True)

            kt_all = consts.tile([128, B, S], BF16)  # [d, b, k]
            qt_all = consts.tile([128, B * GH, S], BF16)  # [d, inst, q]
            v_ext = consts.tile([128, B, NQB, 132], BF16)  # [k, b, kblk, d+1]
            nc.vector.memset(v_ext[:, :, :, 128:129], 1.0)
            for b in range(B):
                nc.sync.dma_start(
                    out=v_ext[:, b, :, 0:128],
                    in_=v_d[b, :, 0, :].rearrange("(n p) d -> p n d", p=128),
                )

            # PSUM: two 3-bank QK page tiles (pool) + 2-bank packed PV acc.
            # Each page is its own pool tile so the Tile framework's
            # dependency tracking works at page granularity: QK of group
            # g+1 must not serialize behind the exp of group g.
            po = psum_pool.tile([128, 2, 512], F32)

            def next_page():
                return psum_pool.tile(
                    [128, PAGE, 512], F32, tag="page", bufs=2, name="pg"
                )

            def load_nat(src_ap):
                t = nat_pool.tile([128, NQB, 128], BF16, tag="nat")
                nc.sync.dma_start(
                    out=t, in_=src_ap.rearrange("(n p) d -> p n d", p=128)
                )
                return t

            # ---- startup: K (2x16 blocks) + Q inst0 (16) via PE transposes
            # packed 8 blocks per psum bank, one wide DVE copy per bank.
            k_nats = [load_nat(k_d[b, :, 0, :]) for b in range(B)]
            q_nat0 = load_nat(q_d[0, :, 0, :])
            startup = [
                [
                    (0, k_nats[0], 0, kt_all[:, 0, 0:1024]),
                    (1, q_nat0, 0, qt_all[:, 0, 0:1024]),
                    (2, k_nats[0], 8, kt_all[:, 0, 1024:2048]),
                ],
                [
                    (0, q_nat0, 8, qt_all[:, 0, 1024:2048]),
                    (1, k_nats[1], 0, kt_all[:, 1, 0:1024]),
                    (2, k_nats[1], 8, kt_all[:, 1, 1024:2048]),
                ],
            ]
            for banks in startup:
                pg = next_page()
                for bk, nat, blk0, dst in banks:
                    for i in range(8):
                        nc.tensor.transpose(
                            pg[:, bk, 64 * i : 64 * (i + 1)].bitcast(BF16),
                            nat[:, blk0 + i, :],
                            ident_bf,
                        )
                    nc.vector.tensor_copy(dst, pg[:, bk, :].bitcast(BF16))

            # pending = (pv_closures, finish_closure) of the previous item
            pending = [None]

            def emit_pending_chunk(frac_done):
                """Emit pending PV closures up to fraction frac_done."""
                if pending[0] is None:
                    return
                pvs, _fin, cursor = pending[0]
                tgt = int(len(pvs) * frac_done + 0.5)
                while cursor[0] < tgt:
                    pvs[cursor[0]]()
                    cursor[0] += 1

            def finish_pending():
                if pending[0] is None:
                    return
                pvs, fin, cursor = pending[0]
                while cursor[0] < len(pvs):
                    pvs[cursor[0]]()
                    cursor[0] += 1
                fin()
                pending[0] = None

            def phase_item(inst, sq, q_nat_next):
                b, g = divmod(inst, GH)
                nxt = inst + 1
                q0 = 512 * sq
                noff = 4 * sq
                kd = 4 * sq

                # group plan: off-diag in chunks of <=3 on alternating
                # pages, then the diagonal on a full page.
                ngroups = (noff + PAGE - 1) // PAGE + 1

                # --- this item's PV bookkeeping (deferred emission)
                totals = [noff + 1 + j for j in range(4)]
                bank_tot = [totals[0] + totals[1], totals[2] + totals[3]]
                bank_cnt = [0, 0]
                pvs = []

                def defer_pv(pt_ap, kk, j):
                    def run(pt_ap=pt_ap, kk=kk, j=j):
                        bk = j // 2
                        off = (j % 2) * 256
                        bank_cnt[bk] += 1
                        nc.tensor.matmul(
                            po[:, bk, off : off + 129],
                            lhsT=pt_ap,
                            rhs=v_ext[:, b, kk, 0:129],
                            start=(bank_cnt[bk] == 1),
                            stop=(bank_cnt[bk] == bank_tot[bk]),
                        )
                    pvs.append(run)

                gi = 0
                ki = 0
                while ki < noff:
                    n = min(PAGE, noff - ki)
                    pg = next_page()
                    for t in range(n):
                        nc.tensor.matmul(
                            pg[:, t, :],
                            lhsT=kt_all[:, b, ts(ki + t, 128)],
                            rhs=qt_all[:, inst, q0 : q0 + 512],
                            start=True,
                            stop=True,
                        )
                    pt = pt_pool.tile([128, PAGE, 512], BF16)
                    nc.scalar.activation(
                        pt[:, 0:n, :],
                        pg[:, 0:n, :],
                        mybir.ActivationFunctionType.Exp,
                        scale=SCALE,
                    )
                    for t in range(n):
                        for j in range(4):
                            defer_pv(pt[:, t, ts(j, 128)], ki + t, j)
                    ki += n
                    gi += 1
                    emit_pending_chunk(gi / ngroups)

                # --- diagonal group on a full page
                pg = next_page()
                # drip next instance's Q transposes into the diag page's
                # bank-2 tail (cols 256:512 f32 = 4 bf16 blocks; the diag
                # QK/ACT only use 0:256 of that bank). Emitted before the
                # diag QKs so they never wait on this item's own exps.
                if q_nat_next is not None:
                    for i in range(4):
                        nc.tensor.transpose(
                            pg[:, 2, 256 + 64 * i : 320 + 64 * i].bitcast(BF16),
                            q_nat_next[:, 4 * sq + i, :],
                            ident_bf,
                        )
                nc.tensor.matmul(
                    pg[:, 0, :],
                    lhsT=kt_all[:, b, ts(kd, 128)],
                    rhs=qt_all[:, inst, q0 : q0 + 512],
                    start=True, stop=True,
                )
                nc.tensor.matmul(
                    pg[:, 1, 0:384],
                    lhsT=kt_all[:, b, ts(kd + 1, 128)],
                    rhs=qt_all[:, inst, q0 + 128 : q0 + 512],
                    start=True, stop=False,
                )
                nc.tensor.matmul(
                    pg[:, 1, 384:512],
                    lhsT=kt_all[:, b, ts(kd + 3, 128)],
                    rhs=qt_all[:, inst, q0 + 384 : q0 + 512],
                    start=False, stop=True,
                )
                nc.tensor.matmul(
                    pg[:, 2, 0:256],
                    lhsT=kt_all[:, b, ts(kd + 2, 128)],
                    rhs=qt_all[:, inst, q0 + 256 : q0 + 512],
                    start=True, stop=True,
                )
                ptd = pt_pool.tile([128, PAGE, 512], BF16)
                nc.scalar.activation(
                    ptd[:, 0:2, :],
                    pg[:, 0:2, :],
                    mybir.ActivationFunctionType.Exp,
                    scale=SCALE,
                )
                nc.scalar.activation(
                    ptd[:, 2, 0:256],
                    pg[:, 2, 0:256],
                    mybir.ActivationFunctionType.Exp,
                    scale=SCALE,
                )
                for sl in (
                    ptd[:, 0, 0:128],
                    ptd[:, 1, 0:128],
                    ptd[:, 1, 384:512],
                    ptd[:, 2, 0:128],
                ):
                    nc.vector.tensor_mul(sl, sl, mask)
                if q_nat_next is not None:
                    nc.vector.tensor_copy(
                        qt_all[:, nxt, q0 : q0 + 512],
                        pg[:, 2, 256:512].bitcast(BF16),
                    )

                for j in range(4):
                    defer_pv(ptd[:, 0, ts(j, 128)], kd, j)
                for j in range(1, 4):
                    defer_pv(ptd[:, 1, ts(j - 1, 128)], kd + 1, j)
                for j in range(2, 4):
                    defer_pv(ptd[:, 2, ts(j - 2, 128)], kd + 2, j)
                defer_pv(ptd[:, 1, 384:512], kd + 3, 3)

                # --- finish previous item (its remaining PV + normalize)
                finish_pending()

                def finish():
                    o_sb = osb_pool.tile([128, 4, 128], F32)
                    for j in range(4):
                        bk, off = j // 2, (j % 2) * 256
                        recip = small_pool.tile([128, 1], F32)
                        nc.vector.reciprocal(
                            recip, po[:, bk, off + 128 : off + 129]
                        )
                        nc.vector.tensor_scalar_mul(
                            o_sb[:, j, :], po[:, bk, off : off + 128], recip
                        )
                    nc.sync.dma_start(
                        out=o_d[b, q0 : q0 + 512, g, :].rearrange(
                            "(n p) d -> p n d", p=128
                        ),
                        in_=o_sb,
                    )

                pending[0] = (pvs, finish, [0])

            for inst in range(B * GH):
                if inst + 1 < B * GH:
                    bn, gn = divmod(inst + 1, GH)
                    q_nat_next = load_nat(q_d[bn, :, gn, :])
                else:
                    q_nat_next = None
                for sq in range(NSB):
                    phase_item(inst, sq, q_nat_next if sq < NSB else None)
            finish_pending()

    nc.finalize()
    return nc


def _to_bf16(x):
    return np.asarray(x, dtype=np.float32).astype(ml_dtypes.bfloat16)


def make_in_maps(query, key, value):
    qb = _to_bf16(query)
    kb = _to_bf16(key)
    vb = _to_bf16(value)
    in_maps = []
    for c in range(NCORES):
        in_maps.append(
            {
                "query": np.ascontiguousarray(qb[:, :, GH * c : GH * (c + 1), :]),
                "key": np.ascontiguousarray(kb[:, :, c : c + 1, :]),
                "value": np.ascontiguousarray(vb[:, :, c : c + 1, :]),
            }
        )
    return in_maps


def kernel(query, key, value):
    nc = build_nc()
    res = run_bass_kernel_spmd(
        nc, make_in_maps(query, key, value), core_ids=list(range(NCORES))
    )
    outs = [np.asarray(res.results[c]["out"]) for c in range(NCORES)]
    return np.concatenate(outs, axis=2).astype(np.float32)


if __name__ == "__main__":
    rng = np.random.default_rng(0)
    q = rng.standard_normal((B, S, H, D), dtype=np.float32)
    k = rng.standard_normal((B, S, HK, D), dtype=np.float32)
    v = rng.standard_normal((B, S, HK, D), dtype=np.float32)
    out = kernel(q, k, v)
    print("out", out.shape, out.dtype, float(np.abs(out).max()))


# revision 17
# speedup vs baseline: 2.5191x; 1.1419x over previous
"""Causal GQA attention (prefill) on 8 TRN2 NeuronCores.

Problem: B=2, S=2048, H=32 query heads, Hk=8 kv heads, D=128, f32 I/O.
Sharding: tensor-parallel over heads -- core c gets query heads [4c, 4c+4)
and kv head c. Attention is fully independent per head: no collectives.

Per-core kernel: 8 instances of causal attention, one per (batch, qhead),
processed as a software-pipelined stream of 32 (instance, superblock)
items. Engine budget drives the design (ScalarE exp and PE matmul are
the two near-saturated engines; the PE is strictly in-order so the
emission order IS the schedule):
  - Q/K/V are pre-cast to bf16 on the host (halves DMA bytes, kills all
    on-device casts). Q^T/K^T layouts are produced by PE transposes
    (bf16 transpose-mode matmuls are LDWEIGHTS-free) whose outputs are
    packed 4-8 blocks per PSUM bank tail and copied back to SBUF in one
    wide DVE copy instead of per-block copies.
  - QK^T is computed per (query-superblock 512, key-block 128) into a
    6-bank PSUM ring organized as two 3-bank pages; exp runs on ScalarE
    as ONE activation per page (up to 3 key blocks = 1536 cols) to
    amortize the ~0.35us fixed cost per activation. Diagonal (ragged)
    blocks are packed into 2.5 banks of one page so no garbage columns
    are ever exp'd.
  - P^T tiles feed PV matmuls as stationary weights; V carries an
    appended ones-column so the softmax denominator accumulates in the
    same PSUM tile (column 128). The 4 PV output slabs (129 cols each)
    are packed 2-per-bank into 2 PSUM banks via first-write start /
    last-write stop flags.
  - Items are phase-shifted: item n's PV matmuls are woven between item
    n+1's QK groups as in-order filler, so ring-recycle and exp-latency
    waits never idle the PE (which would also drop its p-state clock).
  - out = PV / denom via DVE reciprocal + per-partition scalar mul.
"""

import numpy as np
import ml_dtypes

import concourse.bass as bass
import concourse.tile as tile
from concourse import bacc, mybir
from concourse.bass import ts
from concourse.bass_utils import run_bass_kernel_spmd
from concourse.masks import make_identity, make_upper_triangular

B = 2
S = 2048
H = 32
HK = 8
D = 128
NCORES = 8
GH = H // NCORES  # query heads per core (= group size here)
SCALE = 0.08838834764831845  # 1/sqrt(128)

F32 = mybir.dt.float32
BF16 = mybir.dt.bfloat16

NQB = S // 128  # 16 query/key blocks of 128
NSB = 4  # query superblocks of 512
PAGE = 3  # ring page size in banks; ring = 2 pages


def build_nc() -> bass.Bass:
    nc = bacc.Bacc(
        "TRN2", target_bir_lowering=False, debug=False, num_devices=NCORES
    )
    q_d = nc.declare_dram_parameter("query", [B, S, GH, D], BF16, isOutput=False)
    k_d = nc.declare_dram_parameter("key", [B, S, 1, D], BF16, isOutput=False)
    v_d = nc.declare_dram_parameter("value", [B, S, 1, D], BF16, isOutput=False)
    o_d = nc.declare_dram_parameter("out", [B, S, GH, D], F32, isOutput=True)

    with tile.TileContext(nc) as tc:
        with (
            tc.tile_pool(name="consts", bufs=1) as consts,
            tc.tile_pool(name="nat", bufs=4) as nat_pool,
            tc.tile_pool(name="pt", bufs=10) as pt_pool,
            tc.tile_pool(name="osb", bufs=3) as osb_pool,
            tc.tile_pool(name="small", bufs=8) as small_pool,
            tc.tile_pool(name="psum", bufs=1, space="PSUM") as psum_pool,
        ):
            ident_bf = consts.tile([128, 128], BF16)
            make_identity(nc, ident_bf)
            # mask[k, q] = 1 where q >= k (keep), 0 above -> kills k > q.
            mask = consts.tile([128, 128], BF16)
            make_upper_triangular(nc, mask, val=1.0, diag=True)

            kt_all = consts.tile([128, B, S], BF16)  # [d, b, k]
            qt_all = consts.tile([128, B * GH, S], BF16)  # [d, inst, q]
            v_ext = consts.tile([128, B, NQB, 132], BF16)  # [k, b, kblk, d+1]

            # PSUM: two 3-bank QK page tiles (pool) + 2-bank packed PV acc.
            # Each page is its own pool tile so the Tile framework's
            # dependency tracking works at page granularity: QK of group
            # g+1 must not serialize behind the exp of group g.
            po = psum_pool.tile([128, 2, 512], F32)

            def next_page():
                return psum_pool.tile(
                    [128, PAGE, 512], F32, tag="page", bufs=2, name="pg"
                )

            def load_nat(src_ap):
                t = nat_pool.tile([128, NQB, 128], BF16, tag="nat")
                nc.sync.dma_start(
                    out=t, in_=src_ap.rearrange("(n p) d -> p n d", p=128)
                )
                return t

            # ---- startup: K (2x16 blocks) + Q inst0 (16) via PE transposes
            # packed 8 blocks per psum bank, one wide DVE copy per bank.
            # Load order puts k0/q0 first so the first QK starts ASAP;
            # v_ext / k1 / q1 aren't needed until later.
            k_nat0 = load_nat(k_d[0, :, 0, :])
            q_nat0 = load_nat(q_d[0, :, 0, :])
            nc.vector.memset(v_ext[:, :, :, 128:129], 1.0)
            for b in range(B):
                nc.sync.dma_start(
                    out=v_ext[:, b, :, 0:128],
                    in_=v_d[b, :, 0, :].rearrange("(n p) d -> p n d", p=128),
                )
            k_nat1 = load_nat(k_d[1, :, 0, :])
            q_nats = {1: load_nat(q_d[0, :, 1, :])}
            startup = [
                [
                    (0, k_nat0, 0, kt_all[:, 0, 0:1024]),
                    (1, k_nat0, 8, kt_all[:, 0, 1024:2048]),
                    (2, q_nat0, 0, qt_all[:, 0, 0:1024]),
                ],
                [
                    (0, q_nat0, 8, qt_all[:, 0, 1024:2048]),
                    (1, k_nat1, 0, kt_all[:, 1, 0:1024]),
                    (2, k_nat1, 8, kt_all[:, 1, 1024:2048]),
                ],
            ]
            for banks in startup:
                pg = next_page()
                for bk, nat, blk0, dst in banks:
                    for i in range(8):
                        nc.tensor.transpose(
                            pg[:, bk, 64 * i : 64 * (i + 1)].bitcast(BF16),
                            nat[:, blk0 + i, :],
                            ident_bf,
                        )
                    nc.vector.tensor_copy(dst, pg[:, bk, :].bitcast(BF16))

            # pending = (pv_closures, finish_closure) of the previous item
            pending = [None]

            def emit_pending_chunk(frac_done):
                """Emit pending PV closures up to fraction frac_done."""
                if pending[0] is None:
                    return
                pvs, _fin, cursor = pending[0]
                tgt = int(len(pvs) * frac_done + 0.5)
                while cursor[0] < tgt:
                    pvs[cursor[0]]()
                    cursor[0] += 1

            def finish_pending():
                if pending[0] is None:
                    return
                pvs, fin, cursor = pending[0]
                while cursor[0] < len(pvs):
                    pvs[cursor[0]]()
                    cursor[0] += 1
                fin()
                pending[0] = None

            def phase_item(inst, sq, q_nat_next):
                b, g = divmod(inst, GH)
                nxt = inst + 1
                q0 = 512 * sq
                noff = 4 * sq
                kd = 4 * sq

                # group plan: off-diag in chunks of <=3 on alternating
                # pages, then the diagonal on a full page.
                ngroups = (noff + PAGE - 1) // PAGE + 1

                # --- this item's PV bookkeeping (deferred emission)
                totals = [noff + 1 + j for j in range(4)]
                bank_tot = [totals[0] + totals[1], totals[2] + totals[3]]
                bank_cnt = [0, 0]
                pvs = []

                def defer_pv(pt_ap, kk, j):
                    def run(pt_ap=pt_ap, kk=kk, j=j):
                        bk = j // 2
                        off = (j % 2) * 256
                        bank_cnt[bk] += 1
                        nc.tensor.matmul(
                            po[:, bk, off : off + 129],
                            lhsT=pt_ap,
                            rhs=v_ext[:, b, kk, 0:129],
                            start=(bank_cnt[bk] == 1),
                            stop=(bank_cnt[bk] == bank_tot[bk]),
                        )
                    pvs.append(run)

                gi = 0
                ki = 0
                while ki < noff:
                    n = min(PAGE, noff - ki)
                    pg = next_page()
                    for t in range(n):
                        nc.tensor.matmul(
                            pg[:, t, :],
                            lhsT=kt_all[:, b, ts(ki + t, 128)],
                            rhs=qt_all[:, inst, q0 : q0 + 512],
                            start=True,
                            stop=True,
                        )
                    pt = pt_pool.tile([128, PAGE, 512], BF16)
                    nc.scalar.activation(
                        pt[:, 0:n, :],
                        pg[:, 0:n, :],
                        mybir.ActivationFunctionType.Exp,
                        scale=SCALE,
                    )
                    for t in range(n):
                        for j in range(4):
                            defer_pv(pt[:, t, ts(j, 128)], ki + t, j)
                    ki += n
                    gi += 1
                    emit_pending_chunk(gi / ngroups)

                # --- diagonal group on a full page
                pg = next_page()
                # drip next instance's Q transposes into the diag page's
                # bank-2 tail (cols 256:512 f32 = 4 bf16 blocks; the diag
                # QK/ACT only use 0:256 of that bank). Emitted before the
                # diag QKs so they never wait on this item's own exps.
                if q_nat_next is not None:
                    for i in range(4):
                        nc.tensor.transpose(
                            pg[:, 2, 256 + 64 * i : 320 + 64 * i].bitcast(BF16),
                            q_nat_next[:, 4 * sq + i, :],
                            ident_bf,
                        )
                nc.tensor.matmul(
                    pg[:, 0, :],
                    lhsT=kt_all[:, b, ts(kd, 128)],
                    rhs=qt_all[:, inst, q0 : q0 + 512],
                    start=True, stop=True,
                )
                nc.tensor.matmul(
                    pg[:, 1, 0:384],
                    lhsT=kt_all[:, b, ts(kd + 1, 128)],
                    rhs=qt_all[:, inst, q0 + 128 : q0 + 512],
                    start=True, stop=False,
                )
                nc.tensor.matmul(
                    pg[:, 1, 384:512],
                    lhsT=kt_all[:, b, ts(kd + 3, 128)],
                    rhs=qt_all[:, inst, q0 + 384 : q0 + 512],
                    start=False, stop=True,
                )
                nc.tensor.matmul(
                    pg[:, 2, 0:256],
                    lhsT=kt_all[:, b, ts(kd + 2, 128)],
                    rhs=qt_all[:, inst, q0 + 256 : q0 + 512],
                    start=True, stop=True,
                )
                ptd = pt_pool.tile([128, PAGE, 512], BF16)
                nc.scalar.activation(
                    ptd[:, 0:2, :],
                    pg[:, 0:2, :],
                    mybir.ActivationFunctionType.Exp,
                    scale=SCALE,
                )
                nc.scalar.activation(
                    ptd[:, 2, 0:256],
                    pg[:, 2, 0:256],
                    mybir.ActivationFunctionType.Exp,
                    scale=SCALE,
                )
                for sl in (
                    ptd[:, 0, 0:128],
                    ptd[:, 1, 0:128],
                    ptd[:, 1, 384:512],
                    ptd[:, 2, 0:128],
                ):
                    nc.vector.tensor_mul(sl, sl, mask)
                if q_nat_next is not None:
                    nc.vector.tensor_copy(
                        qt_all[:, nxt, q0 : q0 + 512],
                        pg[:, 2, 256:512].bitcast(BF16),
                    )

                for j in range(4):
                    defer_pv(ptd[:, 0, ts(j, 128)], kd, j)
                for j in range(1, 4):
                    defer_pv(ptd[:, 1, ts(j - 1, 128)], kd + 1, j)
                for j in range(2, 4):
                    defer_pv(ptd[:, 2, ts(j - 2, 128)], kd + 2, j)
                defer_pv(ptd[:, 1, 384:512], kd + 3, 3)

                # --- finish previous item (its remaining PV + normalize)
                finish_pending()

                def finish():
                    # batched normalize: slab j sits at po offset j*256,
                    # denominator at col 128 of each slab. One reciprocal
                    # over all 4 denominators + one broadcast multiply.
                    o_sb = osb_pool.tile([128, 4, 128], F32)
                    recip = small_pool.tile([128, 4], F32)
                    base = po[:, :, :]
                    pp = list(base.ap[0])
                    den_ap = bass.AP(
                        base.tensor, base.offset + 128, [pp, [256, 4], [1, 1]]
                    )
                    pv_ap = bass.AP(
                        base.tensor, base.offset, [pp, [256, 4], [1, 128]]
                    )
                    nc.vector.reciprocal(recip, den_ap)
                    rb = recip[:, :]
                    rb_b = bass.AP(
                        rb.tensor, rb.offset, [list(rb.ap[0]), [1, 4], [0, 128]]
                    )
                    ob = o_sb[:, :, :]
                    ob3 = bass.AP(
                        ob.tensor, ob.offset, [list(ob.ap[0]), [128, 4], [1, 128]]
                    )
                    nc.vector.tensor_mul(ob3, pv_ap, rb_b)
                    nc.sync.dma_start(
                        out=o_d[b, q0 : q0 + 512, g, :].rearrange(
                            "(n p) d -> p n d", p=128
                        ),
                        in_=o_sb,
                    )

                pending[0] = (pvs, finish, [0])

            # q_nat for inst+1 is loaded one full instance ahead (during
            # inst-1) so the drip transposes never wait on the DMA.
            for inst in range(B * GH):
                if inst + 2 < B * GH:
                    bn, gn = divmod(inst + 2, GH)
                    q_nats[inst + 2] = load_nat(q_d[bn, :, gn, :])
                q_nat_next = q_nats.get(inst + 1)
                for sq in range(NSB):
                    phase_item(inst, sq, q_nat_next)
            finish_pending()

    nc.finalize()
    return nc


def _to_bf16(x):
    return np.asarray(x, dtype=np.float32).astype(ml_dtypes.bfloat16)


def make_in_maps(query, key, value):
    qb = _to_bf16(query)
    kb = _to_bf16(key)
    vb = _to_bf16(value)
    in_maps = []
    for c in range(NCORES):
        in_maps.append(
            {
                "query": np.ascontiguousarray(qb[:, :, GH * c : GH * (c + 1), :]),
                "key": np.ascontiguousarray(kb[:, :, c : c + 1, :]),
                "value": np.ascontiguousarray(vb[:, :, c : c + 1, :]),
            }
        )
    return in_maps


def kernel(query, key, value):
    nc = build_nc()
    res = run_bass_kernel_spmd(
        nc, make_in_maps(query, key, value), core_ids=list(range(NCORES))
    )
    outs = [np.asarray(res.results[c]["out"]) for c in range(NCORES)]
    return np.concatenate(outs, axis=2).astype(np.float32)


if __name__ == "__main__":
    rng = np.random.default_rng(0)
    q = rng.standard_normal((B, S, H, D), dtype=np.float32)
    k = rng.standard_normal((B, S, HK, D), dtype=np.float32)
    v = rng.standard_normal((B, S, HK, D), dtype=np.float32)
    out = kernel(q, k, v)
    print("out", out.shape, out.dtype, float(np.abs(out).max()))
